# revision 7
# baseline (speedup 1.0000x reference)
"""Trainium2 kernel for BinaryXnorExceptOutliersLinear.

Computes  out = x @ w_sim.T + bias  where
  w_sim = where(outlier_mask, weight, sign(weight) * binary_scale)

Distribution: column-parallel over 8 NeuronCores — weight / outlier_mask /
bias are sharded along out_features (11008 -> 8 x 1376), x is replicated,
each core produces its [8192, 1376] output slice, concatenated on host.

Per-core kernel:
  1. Weight prep (once): DMA weight+mask shard, sign via ACT, scale via DVE
     tensor_scalar, outlier restore via DVE copy_predicated, then PE
     transpose into a SBUF-resident [K=4096, 1376] bf16 wT (88KB/partition).
  2. Main loop over 64 token tiles: gpsimd cast-DMA x f32->bf16, PE
     transposes x tiles (interleaved with the previous tile's matmuls so PE
     stays busy), 3x32 accumulating matmuls per tile (chunks 512/512/352
     over out-features, 32 k-tiles), DVE adds bias on the PSUM->SBUF copy,
     DMA out.
"""

import sys

for _p in ("/opt/trn_rl_repo",):
    if _p not in sys.path:
        sys.path.insert(0, _p)

import numpy as np

import concourse.bass as bass
import concourse.mybir as mybir
from concourse.tile import TileContext
from concourse.bass_utils import run_bass_kernel_spmd
from concourse.masks import make_identity

B, S, DIN, DOUT = 4, 2048, 4096, 11008
M = B * S              # 8192 tokens
NCORES = 8
DSH = DOUT // NCORES   # 1376 out-features per core
K = DIN
KT = K // 128          # 32 k-tiles
CHUNKS = [(0, 512), (512, 512), (1024, 352)]   # out-feature chunks per core

F32 = mybir.dt.float32
BF16 = mybir.dt.bfloat16
U8 = mybir.dt.uint8

MAX_WAITS = 1  # stock walrus: one sem-wait command per instruction


def _split_excess_waits(nc, max_waits: int = MAX_WAITS) -> int:
    """Stock AWS walrus rejects instructions with more than one sem-wait
    ("Too many sync wait commands"). Tile's kernel-tail drain waits on the
    final value of every proc's semaphore. Peel excess waits onto bare
    EventSemaphore stubs placed right before the instruction on the same
    engine (engines run their stream in order, so ordering is preserved)."""
    n_split = 0
    for f in nc.m.functions:
        for blk in f.blocks:
            il = blk.instructions
            out = []
            changed = False
            for inst in il:
                si = inst.sync_info
                waits = list(si.on_wait) if (si and si.on_wait) else []
                if len(waits) > max_waits:
                    changed = True
                    extra, keep = waits[:-max_waits], waits[-max_waits:]
                    for ci, start in enumerate(range(0, len(extra), max_waits)):
                        chunk = extra[start:start + max_waits]
                        stub = mybir.InstEventSemaphore(
                            name=f"{inst.name}_wsplit{ci}", ins=[], outs=[])
                        stub.engine = inst.engine
                        stub.sync_info = mybir.SyncInfo(
                            on_wait=list(chunk), on_update=[])
                        out.append(stub)
                        n_split += 1
                    si.on_wait = keep
                    inst.sync_info = si
                out.append(inst)
            if changed:
                il.clear()
                il.extend(out)
    return n_split


def build_nc(m_tokens: int = M):
    """Build the per-core Bass program (SPMD: same program on all cores)."""
    tok_tiles = m_tokens // 128
    nc = bass.Bass()
    x_h = nc.declare_dram_parameter("x", [m_tokens, K], F32, isOutput=False)
    w_h = nc.declare_dram_parameter("weight", [DSH, K], F32, isOutput=False)
    b_h = nc.declare_dram_parameter("bias", [DSH], F32, isOutput=False)
    mk_h = nc.declare_dram_parameter("outlier_mask", [DSH, K], U8, isOutput=False)
    sc_h = nc.declare_dram_parameter("binary_scale", [1, 1], F32, isOutput=False)
    out_h = nc.declare_dram_parameter("out", [m_tokens, DSH], F32, isOutput=True)

    dout_tiles = [(o, min(128, DSH - o)) for o in range(0, DSH, 128)]

    with TileContext(nc) as tc:
        with tc.tile_pool(name="const", bufs=1) as const_pool:

            identity = const_pool.tile([128, 128], BF16)
            make_identity(nc, identity)
            scale_vec = const_pool.tile([128, 1], F32)
            nc.gpsimd.dma_start(out=scale_vec,
                                in_=sc_h[:, :].to_broadcast((128, 1)))
            bias_rep = const_pool.tile([128, DSH], F32)
            nc.gpsimd.dma_start(
                out=bias_rep,
                in_=b_h[:].rearrange("(a d) -> a d", a=1).to_broadcast((128, DSH)))

            # Resident binarized+transposed weight: [k-in-tile, kt, dout]
            wT = const_pool.tile([128, KT * DSH], BF16)
            wT_r = wT.rearrange("p (kt d) -> p kt d", kt=KT)

            # ---- weight prep ----
            with tc.tile_pool(name="wprep", bufs=2) as wp, \
                 tc.tile_pool(name="wpsum", bufs=2, space="PSUM") as psum_pool:
                for doff, p in dout_tiles:
                    wf = wp.tile([128, K], F32, tag="wf")
                    mk = wp.tile([128, K], U8, tag="mk")
                    sgn = wp.tile([128, K], BF16, tag="sgn")
                    wsb = wp.tile([128, K], BF16, tag="wsb")
                    nc.sync.dma_start(wf[:p], w_h[doff:doff + p, :])
                    nc.sync.dma_start(mk[:p], mk_h[doff:doff + p, :])
                    nc.scalar.sign(sgn[:p], wf[:p])
                    nc.vector.tensor_scalar_mul(wsb[:p], sgn[:p], scale_vec[:p])
                    nc.vector.copy_predicated(wsb[:p], mk[:p], wf[:p])
                    for kt4 in range(KT // 4):
                        psw = psum_pool.tile([128, 512], BF16, tag="psw")
                        for j in range(4):
                            kt = kt4 * 4 + j
                            nc.tensor.transpose(
                                psw[:, j * 128:j * 128 + p],
                                wsb[:p, kt * 128:(kt + 1) * 128],
                                identity[:p, :p])
                        nc.vector.tensor_copy(
                            wT_r[:, kt4 * 4:(kt4 + 1) * 4, doff:doff + p],
                            psw.rearrange("a (j c) -> a j c", j=4)[:, :, :p])

            # ---- main loop (x transposes for tile t interleave with
            #      matmuls of tile t-1 so the PE never idles) ----
            with tc.tile_pool(name="xmain", bufs=2) as xp, \
                 tc.tile_pool(name="mpsum", bufs=2, space="PSUM") as psum_pool:
                xT_prev = None
                for t in range(tok_tiles + 1):
                    if t < tok_tiles:
                        xb = xp.tile([128, K], BF16, tag="xb")
                        nc.gpsimd.dma_start(xb, x_h[t * 128:(t + 1) * 128, :])
                        xT = xp.tile([128, K], BF16, tag="xT")
                    if t >= 1:
                        osb = xp.tile([128, DSH], F32, tag="osb")
                        psos = [psum_pool.tile([128, 512], F32, tag=f"pso{ci}",
                                               name=f"pso{ci}")
                                for ci in range(len(CHUNKS))]
                    psx = None
                    for kt in range(KT):
                        if t < tok_tiles:
                            j = kt % 4
                            if j == 0:
                                psx = psum_pool.tile([128, 512], BF16, tag="psx")
                            nc.tensor.transpose(
                                psx[:, j * 128:(j + 1) * 128],
                                xb[:, kt * 128:(kt + 1) * 128],
                                identity)
                            if j == 3:
                                nc.vector.tensor_copy(
                                    xT[:, (kt - 3) * 128:(kt + 1) * 128], psx)
                        if t >= 1:
                            for ci, (coff, csz) in enumerate(CHUNKS):
                                nc.tensor.matmul(
                                    psos[ci][:, :csz],
                                    xT_prev[:, kt * 128:(kt + 1) * 128],
                                    wT_r[:, kt, coff:coff + csz],
                                    start=(kt == 0), stop=(kt == KT - 1))
                    if t >= 1:
                        for ci, (coff, csz) in enumerate(CHUNKS):
                            nc.vector.tensor_add(
                                osb[:, coff:coff + csz], psos[ci][:, :csz],
                                bias_rep[:, coff:coff + csz])
                        nc.sync.dma_start(
                            out_h[(t - 1) * 128:t * 128, :], osb)
                    xT_prev = xT

    _split_excess_waits(nc)
    return nc


_NC_CACHE = {}


def _get_nc(m_tokens: int = M):
    if m_tokens not in _NC_CACHE:
        _NC_CACHE[m_tokens] = build_nc(m_tokens)
    return _NC_CACHE[m_tokens]


def _make_in_maps(x, weight, bias, outlier_mask, binary_scale):
    m_tokens = x.shape[0] * x.shape[1] if x.ndim == 3 else x.shape[0]
    xf = np.ascontiguousarray(x.reshape(m_tokens, K), dtype=np.float32)
    w = np.ascontiguousarray(weight, dtype=np.float32)
    b = np.ascontiguousarray(bias, dtype=np.float32)
    mk = np.ascontiguousarray(outlier_mask).view(np.uint8)
    sc = np.ascontiguousarray(binary_scale, dtype=np.float32).reshape(1, 1)
    in_maps = []
    for i in range(NCORES):
        sl = slice(i * DSH, (i + 1) * DSH)
        in_maps.append({
            "x": xf,
            "weight": np.ascontiguousarray(w[sl]),
            "bias": np.ascontiguousarray(b[sl]),
            "outlier_mask": np.ascontiguousarray(mk[sl]),
            "binary_scale": sc,
        })
    return in_maps, m_tokens


def run_sharded(x, weight, bias, outlier_mask, binary_scale, trace=False):
    """Run on 8 cores; returns (full_output [M, DOUT] f32, BassKernelResults)."""
    in_maps, m_tokens = _make_in_maps(x, weight, bias, outlier_mask, binary_scale)
    nc = _get_nc(m_tokens)
    res = run_bass_kernel_spmd(nc, in_maps, core_ids=list(range(NCORES)),
                               trace=trace)
    full = np.concatenate([res.results[i]["out"] for i in range(NCORES)], axis=1)
    return full, res


def kernel(x, weight, bias, outlier_mask, binary_scale):
    full, _ = run_sharded(x, weight, bias, outlier_mask, binary_scale)
    return full.reshape(x.shape[0], x.shape[1], DOUT) if x.ndim == 3 else full
